# revision 30
# baseline (speedup 1.0000x reference)
"""Multi-head attention on 8 TRN2 NeuronCores.

Problem: queries [B,N,L,H,E], keys [B,N,S,H,E], values [B,N,S,H,D]
         out[b,n,l,h,:] = softmax(Q[b,n,l,h,:] @ K[b,n,:,h,:]^T / sqrt(E)) @ V[b,n,:,h,:]
with B,N,L,S,H,E,D = 4,7,512,512,8,64,64.

Sharding: head-parallel - core c computes all B*N=28 (b,n) slices for head h=c.

Steady state is ScalarE-paced (exp of 512x512 scores = 2048 elem/lane/slice at
1.2GHz ~= the PE's 8 matmuls at 2.4GHz). Design per slice (fp16 operands):
  1. scoresT chunks [128s, 512l] = K_c^T x Q^T into TWO 2-bank PSUM tiles per
     slice (psA = chunks 0,1; psB = chunks 2,3), tags ping-pong i%2. Tile's
     dependency tracking is per-TILE: with one 4-bank tile, QK(i) waits on the
     copy-out of slice i-2's PV result inside the same tile, serializing
     ACT->PV->copy->QK (~3.3us > the 2.2us ScalarE window) and HAM-oscillating
     the PE. Split tiles break the chain: QK(i+2)'s psA half only waits on
     ACT_A(i); the psB chain has ~0.7us slack over its 2-window budget.
     K chunks are packed alternately on partition halves (c0,c2 on 0:64 /
     c1,c3 on 64:128, Q duplicated on both halves) so consecutive QK matmuls
     hit different PE row groups: LDWEIGHTS overlaps and pairs run
     concurrently in the array (verified: ~0ns start deltas in trace).
  2. exp via two ACTIVATEs per slice: ACT_A (psA) first, ACT_B (psB) second.
  3. po [65, 512] += VA_c^T x attnT_c accumulated into psB's first bank (free
     after ACT_B read). VA = [ones | V] per 128-chunk, so row 0 of po is the
     softmax denominator, rows 1:65 the numerator^T. No on-device
     normalization: ship [denom | num] and divide on host - this removes the
     recip/broadcast/mul chain and its ~15us serialized drain tail.
  4. DVE copies po to SBUF fp16; output DMA goes out on the GpSimd queue so it
     never parks input DMAs on the Sync queue.
"""

import numpy as np

B, N, L, S, H, E, D = 4, 7, 512, 512, 8, 64, 64
NS = B * N          # 28 slices per core
P = 128
SC = S // P         # 4 s-chunks
SCALE = 1.0 / float(np.sqrt(E))

# input tile layout per slice: [128, 1028] =
#   [0:512)     Q^T duplicated: rows 0:64 = [E, L], rows 64:128 = same
#   [512:768)   K^T 2x2: (row 64*(c%2), col 512+128*(c//2)) = chunk c [E, 128]
#   [768:1028)  VA65: 4 chunks x [128, 65] = [ones | V chunk]
QOFF, KOFF, VOFF = 0, 512, 768
TW = 1028
OW = 65             # output rows: 1 denom + 64 numerator

NP = NS // 2        # 14 slice-pairs (DMA batching granularity)
PW = 2 * TW         # 2056 cols per pair tile

_CACHE = {}


def _build_program():
    import concourse.mybir as mybir
    import concourse.tile as tile
    from concourse import bacc
    import concourse.bass as bass

    f32 = mybir.dt.float32
    f16 = mybir.dt.float16
    Exp = mybir.ActivationFunctionType.Exp

    nc = bacc.Bacc("TRN2", target_bir_lowering=False, debug=False)
    inp = nc.dram_tensor("inp", [NP, P, PW], f16, kind="ExternalInput").ap()
    o = nc.dram_tensor("o", [NP, OW, 2 * L], f16, kind="ExternalOutput").ap()

    with tile.TileContext(nc) as tc:
        with (
            tc.tile_pool(name="inpool", bufs=1) as in_pool,
            tc.tile_pool(name="attn", bufs=1) as at_pool,
            tc.tile_pool(name="osb", bufs=1) as osb_pool,
            tc.tile_pool(name="dum", bufs=1) as dum_pool,
            tc.tile_pool(name="ps", bufs=1, space=bass.MemorySpace.PSUM) as ps_pool,
        ):
            # HAM warm-up: ~4.5us of continuous PE busy on a memset tile opens
            # the clock gate (1.2 -> 2.4 GHz) before the steady pipeline.
            warm = in_pool.tile([P, L], f16, tag="warm")
            nc.vector.memset(warm[:], 1.0)
            # Preload the exp table set so the first real ACTIVATE doesn't pay
            # the ~2.7us table load mid-pipeline.
            dummy = dum_pool.tile([1, 2], f32, tag="d0")
            nc.scalar.activation(dummy[:], warm[0:1, 0:2], Exp, scale=SCALE)
            wps = ps_pool.tile([P, 1024], f32, tag="psA0")
            for _ in range(5):
                nc.tensor.matmul(
                    wps[:, 0:L], lhsT=warm[:, 0:P], rhs=warm[:], start=True, stop=True
                )

            in_tiles = {}

            def load(p):
                if p < NP and p not in in_tiles:
                    t = in_pool.tile([P, PW], f16, tag=f"t{p % 5}")
                    nc.sync.dma_start(t[:], inp[p])
                    in_tiles[p] = t

            for p in range(4):
                load(p)

            osb_tiles = {}

            def emit_pv_out(state):
                i, in_t, at, psb = state
                jo = (i % 2) * TW
                # PV accumulates into psB's first bank, free after ACT_B read
                # it. [ones | V] stationary: po row 0 = denominator, rows
                # 1:65 = numerator^T. (Splitting each chunk into concurrent
                # 64-row half-matmuls measured WORSE - interleaved
                # accumulation groups serialize with ~800ns stalls.)
                for c in range(SC):
                    atc = at[c] if isinstance(at, list) else at[:, c * L:(c + 1) * L]
                    nc.tensor.matmul(
                        psb[0:OW, 0:L],
                        lhsT=in_t[:, jo + VOFF + c * OW: jo + VOFF + (c + 1) * OW],
                        rhs=atc,
                        start=(c == 0),
                        stop=(c == SC - 1),
                    )
                # Outputs batch per pair: both slices copy into one osb tile,
                # one DMA per pair (fewer sync events and DMA descriptors).
                if i % 2 == 0:
                    osb_tiles[i // 2] = osb_pool.tile(
                        [OW, 2 * L], f16, tag=f"o{(i // 2) % 3}", name=f"osb{i // 2}"
                    )
                osb = osb_tiles[i // 2]
                nc.vector.tensor_scalar_mul(
                    osb[:, (i % 2) * L:(i % 2 + 1) * L], psb[0:OW, 0:L], 1.0
                )
                if i % 2 == 1:
                    nc.gpsimd.dma_start(o[i // 2], osb_tiles.pop(i // 2)[:])

            pend = []
            for i in range(NS):
                load(i // 2 + 4)
                in_t = in_tiles[i // 2] if i % 2 == 0 else in_tiles.pop(i // 2)
                jo = (i % 2) * TW
                psa = ps_pool.tile([P, 1024], f32, tag=f"psA{i % 2}")
                psb = ps_pool.tile([P, 1024], f32, tag=f"psB{i % 2}")
                # Chunk c: K at rows 64*(c%2), col KOFF+128*(c//2); the order
                # alternates row halves so LDWEIGHTS overlaps the running MM
                # and each (even, odd) pair runs concurrently in the array.
                for c in (0, 1, 2, 3):
                    ps = psa if c < 2 else psb
                    ro = 64 * (c % 2)
                    co = jo + KOFF + P * (c // 2)
                    nc.tensor.matmul(
                        ps[:, (c % 2) * L:(c % 2 + 1) * L],
                        lhsT=in_t[ro:ro + E, co:co + P],
                        rhs=in_t[ro:ro + E, jo + QOFF:jo + QOFF + L],
                        start=True,
                        stop=True,
                    )
                if i < NS - 1:
                    at = at_pool.tile([P, 2048], f16, tag=f"at{i % 4}")
                    nc.scalar.activation(at[:, 0:1024], psa[:], Exp, scale=SCALE)
                    nc.scalar.activation(at[:, 1024:2048], psb[:], Exp, scale=SCALE)
                    if i in (0, 2):
                        # Pace ScalarE during the pipeline fill: windows 0-3
                        # run ~40ns/slice faster than the PE's sustainable
                        # pace; without this throttle the PE falls behind and
                        # the pipeline hunts at +220ns/window for slices 5-9.
                        nc.scalar.activation(dummy[:], warm[0:1, 0:2], Exp, scale=SCALE)
                    pend.append((i, in_t, at, psb))
                else:
                    # Last slice: psb's two chunks get separate ACTs into
                    # separate tiles (tile deps are tile-granular) so the PV
                    # matmuls chase the exp chunk-by-chunk, and PV targets
                    # psa (free right after ACT_A) instead of psb (whose
                    # readers finish last) - shortens the drain tail ~0.7us.
                    ata = at_pool.tile([P, 1024], f16, tag="atl0", name="atl0")
                    nc.scalar.activation(ata[:], psa[:], Exp, scale=SCALE)
                    at = [ata[:, 0:L], ata[:, L:2 * L]]
                    for c in (2, 3):
                        atc = at_pool.tile([P, L], f16, tag=f"atl{c}", name=f"atl{c}")
                        nc.scalar.activation(
                            atc[:], psb[:, (c % 2) * L:(c % 2 + 1) * L], Exp, scale=SCALE
                        )
                        at.append(atc[:])
                    pend.append((i, in_t, at, psa))
                if len(pend) > 1:
                    emit_pv_out(pend.pop(0))
            for state in pend:
                emit_pv_out(state)
    nc.compile()
    return nc


def _prep_inputs(queries, keys, values):
    """Pack per-core fp16 inputs. Core c gets head h=c."""
    q = np.asarray(queries, dtype=np.float32)
    k = np.asarray(keys, dtype=np.float32)
    v = np.asarray(values, dtype=np.float32)

    # Q^T per slice [H, NS, E, L], duplicated onto both partition halves
    qt = np.ascontiguousarray(q.transpose(3, 0, 1, 4, 2)).reshape(H, NS, E, L)
    qd = np.concatenate([qt, qt], axis=2)                      # [H, NS, 128, 512]

    # K^T [H, NS, E, S] -> 2x2 chunk grid [H, NS, 128, 256]
    kt = np.ascontiguousarray(k.transpose(3, 0, 1, 4, 2)).reshape(H, NS, E, S)
    kc = kt.reshape(H, NS, E, 2, 2, P)                         # [..., c//2, c%2, 128]
    # karr[rows 64*(c%2), cols 128*(c//2)] = chunk c
    karr = np.ascontiguousarray(
        kc.transpose(0, 1, 4, 2, 3, 5)                         # [H,NS, c%2, E, c//2, 128]
    ).reshape(H, NS, P, 2 * P)

    # VA65: [H, NS, 4, 128, 65] = [ones | V chunk] -> [H, NS, 128, 260]
    vt = v.transpose(3, 0, 1, 2, 4).reshape(H, NS, SC, P, D)
    va = np.zeros((H, NS, SC, P, OW), dtype=np.float32)
    va[..., 0] = 1.0
    va[..., 1:] = vt
    va = np.ascontiguousarray(va.transpose(0, 1, 3, 2, 4)).reshape(H, NS, P, SC * OW)

    inp = np.concatenate([qd, karr, va], axis=-1).astype(np.float16)
    # batch slice pairs side by side: [H, NP, 128, 2*TW]
    inp = np.ascontiguousarray(
        inp.reshape(H, NP, 2, P, TW).transpose(0, 1, 3, 2, 4)
    ).reshape(H, NP, P, PW)
    return [{"inp": inp[c]} for c in range(H)]


def _run(in_maps, trace=False, tmpdir=None):
    from concourse.bass_utils import run_bass_kernel_spmd

    if "nc" not in _CACHE:
        _CACHE["nc"] = _build_program()
    kwargs = {}
    if tmpdir is not None:
        kwargs["tmpdir"] = tmpdir
    return run_bass_kernel_spmd(
        _CACHE["nc"], in_maps, core_ids=list(range(H)), trace=trace, **kwargs
    )


def kernel(queries, keys, values, _trace=False, _results_out=None, _tmpdir=None):
    in_maps = _prep_inputs(queries, keys, values)
    res = _run(in_maps, trace=_trace, tmpdir=_tmpdir)
    if _results_out is not None:
        _results_out.append(res)
    # res.results[c]["o"]: [NP, 65, 2L] fp16 (slice pair side by side):
    # row 0 = denom, rows 1:65 = numerator^T
    raw = np.stack([res.results[c]["o"] for c in range(H)], axis=0).astype(np.float32)
    raw = raw.reshape(H, NP, OW, 2, L).transpose(0, 1, 3, 2, 4).reshape(H, NS, OW, L)
    num = raw[:, :, 1:, :]                     # [H, NS, D, L]
    den = raw[:, :, 0:1, :]                    # [H, NS, 1, L]
    out = (num / den).reshape(H, B, N, D, L).transpose(1, 2, 4, 0, 3)
    return np.ascontiguousarray(out)


# revision 31
# speedup vs baseline: 1.1731x; 1.1731x over previous
"""Multi-head attention on 8 TRN2 NeuronCores.

Problem: queries [B,N,L,H,E], keys [B,N,S,H,E], values [B,N,S,H,D]
         out[b,n,l,h,:] = softmax(Q[b,n,l,h,:] @ K[b,n,:,h,:]^T / sqrt(E)) @ V[b,n,:,h,:]
with B,N,L,S,H,E,D = 4,7,512,512,8,64,64.

Sharding: head-parallel - core c computes all B*N=28 (b,n) slices for head h=c.

Steady state is ScalarE-paced (exp of 512x512 scores = 2048 elem/lane/slice at
1.2GHz ~= the PE's 8 matmuls at 2.4GHz). Design per slice (fp16 operands):
  1. scoresT chunks [128s, 512l] = K_c^T x Q^T into TWO 2-bank PSUM tiles per
     slice (psA = chunks 0,1; psB = chunks 2,3), tags ping-pong i%2. Tile's
     dependency tracking is per-TILE: with one 4-bank tile, QK(i) waits on the
     copy-out of slice i-2's PV result inside the same tile, serializing
     ACT->PV->copy->QK (~3.3us > the 2.2us ScalarE window) and HAM-oscillating
     the PE. Split tiles break the chain: QK(i+2)'s psA half only waits on
     ACT_A(i); the psB chain has ~0.7us slack over its 2-window budget.
     K chunks are packed alternately on partition halves (c0,c2 on 0:64 /
     c1,c3 on 64:128, Q duplicated on both halves) so consecutive QK matmuls
     hit different PE row groups: LDWEIGHTS overlaps and pairs run
     concurrently in the array (verified: ~0ns start deltas in trace).
  2. exp via two ACTIVATEs per slice: ACT_A (psA) first, ACT_B (psB) second.
  3. po [65, 512] += VA_c^T x attnT_c accumulated into psB's first bank (free
     after ACT_B read). VA = [ones | V] per 128-chunk, so row 0 of po is the
     softmax denominator, rows 1:65 the numerator^T. No on-device
     normalization: ship [denom | num] and divide on host - this removes the
     recip/broadcast/mul chain and its ~15us serialized drain tail.
  4. DVE copies po to SBUF fp16; output DMA goes out on the GpSimd queue so it
     never parks input DMAs on the Sync queue.
"""

import numpy as np

B, N, L, S, H, E, D = 4, 7, 512, 512, 8, 64, 64
NS = B * N          # 28 slices per core
P = 128
SC = S // P         # 4 s-chunks
SCALE = 1.0 / float(np.sqrt(E))

# input tile layout per slice: [128, 1028] =
#   [0:512)     Q^T duplicated: rows 0:64 = [E, L], rows 64:128 = same
#   [512:768)   K^T 2x2: (row 64*(c%2), col 512+128*(c//2)) = chunk c [E, 128]
#   [768:1028)  VA65: 4 chunks x [128, 65] = [ones | V chunk]
QOFF, KOFF, VOFF = 0, 512, 768
TW = 1028
OW = 65             # output rows: 1 denom + 64 numerator

NP = NS // 2        # 14 slice-pairs (DMA batching granularity)
PW = 2 * TW         # 2056 cols per pair tile

_CACHE = {}


def _build_program():
    import concourse.mybir as mybir
    import concourse.tile as tile
    from concourse import bacc
    import concourse.bass as bass

    f32 = mybir.dt.float32
    f16 = mybir.dt.float16
    Exp = mybir.ActivationFunctionType.Exp

    nc = bacc.Bacc("TRN2", target_bir_lowering=False, debug=False)
    inp = nc.dram_tensor("inp", [NP, P, PW], f16, kind="ExternalInput").ap()
    o = nc.dram_tensor("o", [NP, OW, 2 * L], f16, kind="ExternalOutput").ap()

    with tile.TileContext(nc) as tc:
        with (
            tc.tile_pool(name="inpool", bufs=1) as in_pool,
            tc.tile_pool(name="attn", bufs=1) as at_pool,
            tc.tile_pool(name="osb", bufs=1) as osb_pool,
            tc.tile_pool(name="dum", bufs=1) as dum_pool,
            tc.tile_pool(name="ps", bufs=1, space=bass.MemorySpace.PSUM) as ps_pool,
        ):
            # HAM warm-up: ~4.5us of continuous PE busy on a memset tile opens
            # the clock gate (1.2 -> 2.4 GHz) before the steady pipeline.
            warm = in_pool.tile([P, L], f16, tag="warm")
            nc.vector.memset(warm[:], 1.0)
            # Preload the exp table set so the first real ACTIVATE doesn't pay
            # the ~2.7us table load mid-pipeline.
            dummy = dum_pool.tile([1, 2], f32, tag="d0")
            nc.scalar.activation(dummy[:], warm[0:1, 0:2], Exp, scale=SCALE)
            wps = ps_pool.tile([P, 1024], f32, tag="psA0")
            for _ in range(5):
                nc.tensor.matmul(
                    wps[:, 0:L], lhsT=warm[:, 0:P], rhs=warm[:], start=True, stop=True
                )

            in_tiles = {}

            def load(p):
                if p < NP and p not in in_tiles:
                    t = in_pool.tile([P, PW], f16, tag=f"t{p % 7}")
                    nc.sync.dma_start(t[:], inp[p])
                    in_tiles[p] = t

            for p in range(4):
                load(p)

            osb_tiles = {}

            def emit_pv_out(state):
                i, in_t, at, psb = state
                jo = (i % 2) * TW
                # PV accumulates into psB's first bank, free after ACT_B read
                # it. [ones | V] stationary: po row 0 = denominator, rows
                # 1:65 = numerator^T. (Splitting each chunk into concurrent
                # 64-row half-matmuls measured WORSE - interleaved
                # accumulation groups serialize with ~800ns stalls.)
                for c in range(SC):
                    atc = at[c] if isinstance(at, list) else at[:, c * L:(c + 1) * L]
                    nc.tensor.matmul(
                        psb[0:OW, 0:L],
                        lhsT=in_t[:, jo + VOFF + c * OW: jo + VOFF + (c + 1) * OW],
                        rhs=atc,
                        start=(c == 0),
                        stop=(c == SC - 1),
                    )
                # Outputs batch per pair: both slices copy into one osb tile,
                # one DMA per pair (fewer sync events and DMA descriptors).
                if i % 2 == 0:
                    osb_tiles[i // 2] = osb_pool.tile(
                        [OW, 2 * L], f16, tag=f"o{(i // 2) % 7}", name=f"osb{i // 2}"
                    )
                osb = osb_tiles[i // 2]
                nc.vector.tensor_scalar_mul(
                    osb[:, (i % 2) * L:(i % 2 + 1) * L], psb[0:OW, 0:L], 1.0
                )
                if i % 2 == 1:
                    nc.gpsimd.dma_start(o[i // 2], osb_tiles.pop(i // 2)[:])

            pend = []
            for i in range(NS):
                load(i // 2 + 4)
                in_t = in_tiles[i // 2] if i % 2 == 0 else in_tiles.pop(i // 2)
                jo = (i % 2) * TW
                psa = ps_pool.tile([P, 1024], f32, tag=f"psA{i % 2}")
                psb = ps_pool.tile([P, 1024], f32, tag=f"psB{i % 2}")
                # Chunk c: K at rows 64*(c%2), col KOFF+128*(c//2); the order
                # alternates row halves so LDWEIGHTS overlaps the running MM
                # and each (even, odd) pair runs concurrently in the array.
                for c in (0, 1, 2, 3):
                    ps = psa if c < 2 else psb
                    ro = 64 * (c % 2)
                    co = jo + KOFF + P * (c // 2)
                    nc.tensor.matmul(
                        ps[:, (c % 2) * L:(c % 2 + 1) * L],
                        lhsT=in_t[ro:ro + E, co:co + P],
                        rhs=in_t[ro:ro + E, jo + QOFF:jo + QOFF + L],
                        start=True,
                        stop=True,
                    )
                if i < NS - 1:
                    at = at_pool.tile([P, 2048], f16, tag=f"at{i % 4}")
                    nc.scalar.activation(at[:, 0:1024], psa[:], Exp, scale=SCALE)
                    nc.scalar.activation(at[:, 1024:2048], psb[:], Exp, scale=SCALE)
                    if i in (0, 2):
                        # Pace ScalarE during the pipeline fill: windows 0-3
                        # run ~40ns/slice faster than the PE's sustainable
                        # pace; without this throttle the PE falls behind and
                        # the pipeline hunts at +220ns/window for slices 5-9.
                        nc.scalar.activation(dummy[:], warm[0:1, 0:2], Exp, scale=SCALE)
                    pend.append((i, in_t, at, psb))
                else:
                    # Last slice: psb's two chunks get separate ACTs into
                    # separate tiles (tile deps are tile-granular) so the PV
                    # matmuls chase the exp chunk-by-chunk, and PV targets
                    # psa (free right after ACT_A) instead of psb (whose
                    # readers finish last) - shortens the drain tail ~0.7us.
                    ata = at_pool.tile([P, 1024], f16, tag="atl0", name="atl0")
                    nc.scalar.activation(ata[:], psa[:], Exp, scale=SCALE)
                    at = [ata[:, 0:L], ata[:, L:2 * L]]
                    for c in (2, 3):
                        atc = at_pool.tile([P, L], f16, tag=f"atl{c}", name=f"atl{c}")
                        nc.scalar.activation(
                            atc[:], psb[:, (c % 2) * L:(c % 2 + 1) * L], Exp, scale=SCALE
                        )
                        at.append(atc[:])
                    pend.append((i, in_t, at, psa))
                if len(pend) > 1:
                    emit_pv_out(pend.pop(0))
            for state in pend:
                emit_pv_out(state)
    nc.compile()
    return nc


def _prep_inputs(queries, keys, values):
    """Pack per-core fp16 inputs. Core c gets head h=c."""
    q = np.asarray(queries, dtype=np.float32)
    k = np.asarray(keys, dtype=np.float32)
    v = np.asarray(values, dtype=np.float32)

    # Q^T per slice [H, NS, E, L], duplicated onto both partition halves
    qt = np.ascontiguousarray(q.transpose(3, 0, 1, 4, 2)).reshape(H, NS, E, L)
    qd = np.concatenate([qt, qt], axis=2)                      # [H, NS, 128, 512]

    # K^T [H, NS, E, S] -> 2x2 chunk grid [H, NS, 128, 256]
    kt = np.ascontiguousarray(k.transpose(3, 0, 1, 4, 2)).reshape(H, NS, E, S)
    kc = kt.reshape(H, NS, E, 2, 2, P)                         # [..., c//2, c%2, 128]
    # karr[rows 64*(c%2), cols 128*(c//2)] = chunk c
    karr = np.ascontiguousarray(
        kc.transpose(0, 1, 4, 2, 3, 5)                         # [H,NS, c%2, E, c//2, 128]
    ).reshape(H, NS, P, 2 * P)

    # VA65: [H, NS, 4, 128, 65] = [ones | V chunk] -> [H, NS, 128, 260]
    vt = v.transpose(3, 0, 1, 2, 4).reshape(H, NS, SC, P, D)
    va = np.zeros((H, NS, SC, P, OW), dtype=np.float32)
    va[..., 0] = 1.0
    va[..., 1:] = vt
    va = np.ascontiguousarray(va.transpose(0, 1, 3, 2, 4)).reshape(H, NS, P, SC * OW)

    inp = np.concatenate([qd, karr, va], axis=-1).astype(np.float16)
    # batch slice pairs side by side: [H, NP, 128, 2*TW]
    inp = np.ascontiguousarray(
        inp.reshape(H, NP, 2, P, TW).transpose(0, 1, 3, 2, 4)
    ).reshape(H, NP, P, PW)
    return [{"inp": inp[c]} for c in range(H)]


def _run(in_maps, trace=False, tmpdir=None):
    from concourse.bass_utils import run_bass_kernel_spmd

    if "nc" not in _CACHE:
        _CACHE["nc"] = _build_program()
    kwargs = {}
    if tmpdir is not None:
        kwargs["tmpdir"] = tmpdir
    return run_bass_kernel_spmd(
        _CACHE["nc"], in_maps, core_ids=list(range(H)), trace=trace, **kwargs
    )


def kernel(queries, keys, values, _trace=False, _results_out=None, _tmpdir=None):
    in_maps = _prep_inputs(queries, keys, values)
    res = _run(in_maps, trace=_trace, tmpdir=_tmpdir)
    if _results_out is not None:
        _results_out.append(res)
    # res.results[c]["o"]: [NP, 65, 2L] fp16 (slice pair side by side):
    # row 0 = denom, rows 1:65 = numerator^T
    raw = np.stack([res.results[c]["o"] for c in range(H)], axis=0).astype(np.float32)
    raw = raw.reshape(H, NP, OW, 2, L).transpose(0, 1, 3, 2, 4).reshape(H, NS, OW, L)
    num = raw[:, :, 1:, :]                     # [H, NS, D, L]
    den = raw[:, :, 0:1, :]                    # [H, NS, 1, L]
    out = (num / den).reshape(H, B, N, D, L).transpose(1, 2, 4, 0, 3)
    return np.ascontiguousarray(out)


# revision 32
# speedup vs baseline: 1.1755x; 1.0021x over previous
"""Multi-head attention on 8 TRN2 NeuronCores.

Problem: queries [B,N,L,H,E], keys [B,N,S,H,E], values [B,N,S,H,D]
         out[b,n,l,h,:] = softmax(Q[b,n,l,h,:] @ K[b,n,:,h,:]^T / sqrt(E)) @ V[b,n,:,h,:]
with B,N,L,S,H,E,D = 4,7,512,512,8,64,64.

Sharding: head-parallel - core c computes all B*N=28 (b,n) slices for head h=c.

Steady state is ScalarE-paced (exp of 512x512 scores = 2048 elem/lane/slice at
1.2GHz ~= the PE's 8 matmuls at 2.4GHz). Design per slice (fp16 operands):
  1. scoresT chunks [128s, 512l] = K_c^T x Q^T into TWO 2-bank PSUM tiles per
     slice (psA = chunks 0,1; psB = chunks 2,3), tags ping-pong i%2. Tile's
     dependency tracking is per-TILE: with one 4-bank tile, QK(i) waits on the
     copy-out of slice i-2's PV result inside the same tile, serializing
     ACT->PV->copy->QK (~3.3us > the 2.2us ScalarE window) and HAM-oscillating
     the PE. Split tiles break the chain: QK(i+2)'s psA half only waits on
     ACT_A(i); the psB chain has ~0.7us slack over its 2-window budget.
     K chunks are packed alternately on partition halves (c0,c2 on 0:64 /
     c1,c3 on 64:128, Q duplicated on both halves) so consecutive QK matmuls
     hit different PE row groups: LDWEIGHTS overlaps and pairs run
     concurrently in the array (verified: ~0ns start deltas in trace).
  2. exp via two ACTIVATEs per slice: ACT_A (psA) first, ACT_B (psB) second.
  3. po [65, 512] += VA_c^T x attnT_c accumulated into psB's first bank (free
     after ACT_B read). VA = [ones | V] per 128-chunk, so row 0 of po is the
     softmax denominator, rows 1:65 the numerator^T. No on-device
     normalization: ship [denom | num] and divide on host - this removes the
     recip/broadcast/mul chain and its ~15us serialized drain tail.
  4. DVE copies po to SBUF fp16; output DMA goes out on the GpSimd queue so it
     never parks input DMAs on the Sync queue.
"""

import numpy as np

B, N, L, S, H, E, D = 4, 7, 512, 512, 8, 64, 64
NS = B * N          # 28 slices per core
P = 128
SC = S // P         # 4 s-chunks
SCALE = 1.0 / float(np.sqrt(E))

# input tile layout per slice: [128, 1028] =
#   [0:512)     Q^T duplicated: rows 0:64 = [E, L], rows 64:128 = same
#   [512:768)   K^T 2x2: (row 64*(c%2), col 512+128*(c//2)) = chunk c [E, 128]
#   [768:1028)  VA65: 4 chunks x [128, 65] = [ones | V chunk]
QOFF, KOFF, VOFF = 0, 512, 768
TW = 1028
OW = 65             # output rows: 1 denom + 64 numerator

NP = NS // 2        # 14 slice-pairs (DMA batching granularity)
PW = 2 * TW         # 2056 cols per pair tile

_CACHE = {}


def _build_program():
    import concourse.mybir as mybir
    import concourse.tile as tile
    from concourse import bacc
    import concourse.bass as bass

    f32 = mybir.dt.float32
    f16 = mybir.dt.float16
    Exp = mybir.ActivationFunctionType.Exp

    nc = bacc.Bacc("TRN2", target_bir_lowering=False, debug=False)
    inp = nc.dram_tensor("inp", [NP, P, PW], f16, kind="ExternalInput").ap()
    o = nc.dram_tensor("o", [NP, OW, 2 * L], f16, kind="ExternalOutput").ap()

    with tile.TileContext(nc) as tc:
        with (
            tc.tile_pool(name="inpool", bufs=1) as in_pool,
            tc.tile_pool(name="attn", bufs=1) as at_pool,
            tc.tile_pool(name="osb", bufs=1) as osb_pool,
            tc.tile_pool(name="dum", bufs=1) as dum_pool,
            tc.tile_pool(name="ps", bufs=1, space=bass.MemorySpace.PSUM) as ps_pool,
        ):
            # HAM warm-up: ~4.5us of continuous PE busy on a memset tile opens
            # the clock gate (1.2 -> 2.4 GHz) before the steady pipeline.
            warm = in_pool.tile([P, L], f16, tag="warm")
            nc.vector.memset(warm[:], 1.0)
            # Preload the exp table set so the first real ACTIVATE doesn't pay
            # the ~2.7us table load mid-pipeline.
            dummy = dum_pool.tile([1, 2], f32, tag="d0")
            nc.scalar.activation(dummy[:], warm[0:1, 0:2], Exp, scale=SCALE)
            wps = ps_pool.tile([P, 1024], f32, tag="psA0")
            for _ in range(5):
                nc.tensor.matmul(
                    wps[:, 0:L], lhsT=warm[:, 0:P], rhs=warm[:], start=True, stop=True
                )

            in_tiles = {}

            def load(p):
                if p < NP and p not in in_tiles:
                    t = in_pool.tile([P, PW], f16, tag=f"t{p % 5}")
                    nc.sync.dma_start(t[:], inp[p])
                    in_tiles[p] = t

            for p in range(4):
                load(p)

            osb_tiles = {}

            def emit_pv_out(state):
                i, in_t, at, psb = state
                jo = (i % 2) * TW
                # PV accumulates into psB's first bank, free after ACT_B read
                # it. [ones | V] stationary: po row 0 = denominator, rows
                # 1:65 = numerator^T. (Splitting each chunk into concurrent
                # 64-row half-matmuls measured WORSE - interleaved
                # accumulation groups serialize with ~800ns stalls.)
                for c in range(SC):
                    atc = at[c] if isinstance(at, list) else at[:, c * L:(c + 1) * L]
                    nc.tensor.matmul(
                        psb[0:OW, 0:L],
                        lhsT=in_t[:, jo + VOFF + c * OW: jo + VOFF + (c + 1) * OW],
                        rhs=atc,
                        start=(c == 0),
                        stop=(c == SC - 1),
                    )
                # Outputs batch per pair: both slices copy into one osb tile,
                # one DMA per pair (fewer sync events and DMA descriptors).
                if i % 2 == 0:
                    osb_tiles[i // 2] = osb_pool.tile(
                        [OW, 2 * L], f16, tag=f"o{(i // 2) % 3}", name=f"osb{i // 2}"
                    )
                osb = osb_tiles[i // 2]
                nc.vector.tensor_scalar_mul(
                    osb[:, (i % 2) * L:(i % 2 + 1) * L], psb[0:OW, 0:L], 1.0
                )
                if i % 2 == 1:
                    nc.gpsimd.dma_start(o[i // 2], osb_tiles.pop(i // 2)[:])

            pend = []
            for i in range(NS):
                load(i // 2 + 4)
                in_t = in_tiles[i // 2] if i % 2 == 0 else in_tiles.pop(i // 2)
                jo = (i % 2) * TW
                psa = ps_pool.tile([P, 1024], f32, tag=f"psA{i % 2}")
                psb = ps_pool.tile([P, 1024], f32, tag=f"psB{i % 2}")
                # Chunk c: K at rows 64*(c%2), col KOFF+128*(c//2); the order
                # alternates row halves so LDWEIGHTS overlaps the running MM
                # and each (even, odd) pair runs concurrently in the array.
                for c in (0, 1, 2, 3):
                    ps = psa if c < 2 else psb
                    ro = 64 * (c % 2)
                    co = jo + KOFF + P * (c // 2)
                    nc.tensor.matmul(
                        ps[:, (c % 2) * L:(c % 2 + 1) * L],
                        lhsT=in_t[ro:ro + E, co:co + P],
                        rhs=in_t[ro:ro + E, jo + QOFF:jo + QOFF + L],
                        start=True,
                        stop=True,
                    )
                if i < NS - 1:
                    at = at_pool.tile([P, 2048], f16, tag=f"at{i % 4}")
                    nc.scalar.activation(at[:, 0:1024], psa[:], Exp, scale=SCALE)
                    nc.scalar.activation(at[:, 1024:2048], psb[:], Exp, scale=SCALE)
                    if i in (0, 2):
                        # Pace ScalarE during the pipeline fill: windows 0-3
                        # run ~40ns/slice faster than the PE's sustainable
                        # pace; without this throttle the PE falls behind and
                        # the pipeline hunts at +220ns/window for slices 5-9.
                        nc.scalar.activation(dummy[:], warm[0:1, 0:2], Exp, scale=SCALE)
                    pend.append((i, in_t, at, psb))
                else:
                    # Last slice: psb's two chunks get separate ACTs into
                    # separate tiles (tile deps are tile-granular) so the PV
                    # matmuls chase the exp chunk-by-chunk, and PV targets
                    # psa (free right after ACT_A) instead of psb (whose
                    # readers finish last) - shortens the drain tail ~0.7us.
                    ata = at_pool.tile([P, 1024], f16, tag="atl0", name="atl0")
                    nc.scalar.activation(ata[:], psa[:], Exp, scale=SCALE)
                    at = [ata[:, 0:L], ata[:, L:2 * L]]
                    for c in (2, 3):
                        atc = at_pool.tile([P, L], f16, tag=f"atl{c}", name=f"atl{c}")
                        nc.scalar.activation(
                            atc[:], psb[:, (c % 2) * L:(c % 2 + 1) * L], Exp, scale=SCALE
                        )
                        at.append(atc[:])
                    pend.append((i, in_t, at, psa))
                if len(pend) > 1:
                    emit_pv_out(pend.pop(0))
            for state in pend:
                emit_pv_out(state)
    nc.compile()
    return nc


def _prep_inputs(queries, keys, values):
    """Pack per-core fp16 inputs. Core c gets head h=c."""
    q = np.asarray(queries, dtype=np.float32)
    k = np.asarray(keys, dtype=np.float32)
    v = np.asarray(values, dtype=np.float32)

    # Q^T per slice [H, NS, E, L], duplicated onto both partition halves
    qt = np.ascontiguousarray(q.transpose(3, 0, 1, 4, 2)).reshape(H, NS, E, L)
    qd = np.concatenate([qt, qt], axis=2)                      # [H, NS, 128, 512]

    # K^T [H, NS, E, S] -> 2x2 chunk grid [H, NS, 128, 256]
    kt = np.ascontiguousarray(k.transpose(3, 0, 1, 4, 2)).reshape(H, NS, E, S)
    kc = kt.reshape(H, NS, E, 2, 2, P)                         # [..., c//2, c%2, 128]
    # karr[rows 64*(c%2), cols 128*(c//2)] = chunk c
    karr = np.ascontiguousarray(
        kc.transpose(0, 1, 4, 2, 3, 5)                         # [H,NS, c%2, E, c//2, 128]
    ).reshape(H, NS, P, 2 * P)

    # VA65: [H, NS, 4, 128, 65] = [ones | V chunk] -> [H, NS, 128, 260]
    vt = v.transpose(3, 0, 1, 2, 4).reshape(H, NS, SC, P, D)
    va = np.zeros((H, NS, SC, P, OW), dtype=np.float32)
    va[..., 0] = 1.0
    va[..., 1:] = vt
    va = np.ascontiguousarray(va.transpose(0, 1, 3, 2, 4)).reshape(H, NS, P, SC * OW)

    inp = np.concatenate([qd, karr, va], axis=-1).astype(np.float16)
    # batch slice pairs side by side: [H, NP, 128, 2*TW]
    inp = np.ascontiguousarray(
        inp.reshape(H, NP, 2, P, TW).transpose(0, 1, 3, 2, 4)
    ).reshape(H, NP, P, PW)
    return [{"inp": inp[c]} for c in range(H)]


def _run(in_maps, trace=False, tmpdir=None):
    from concourse.bass_utils import run_bass_kernel_spmd

    if "nc" not in _CACHE:
        _CACHE["nc"] = _build_program()
    kwargs = {}
    if tmpdir is not None:
        kwargs["tmpdir"] = tmpdir
    return run_bass_kernel_spmd(
        _CACHE["nc"], in_maps, core_ids=list(range(H)), trace=trace, **kwargs
    )


def kernel(queries, keys, values, _trace=False, _results_out=None, _tmpdir=None):
    in_maps = _prep_inputs(queries, keys, values)
    res = _run(in_maps, trace=_trace, tmpdir=_tmpdir)
    if _results_out is not None:
        _results_out.append(res)
    # res.results[c]["o"]: [NP, 65, 2L] fp16 (slice pair side by side):
    # row 0 = denom, rows 1:65 = numerator^T
    raw = np.stack([res.results[c]["o"] for c in range(H)], axis=0).astype(np.float32)
    raw = raw.reshape(H, NP, OW, 2, L).transpose(0, 1, 3, 2, 4).reshape(H, NS, OW, L)
    num = raw[:, :, 1:, :]                     # [H, NS, D, L]
    den = raw[:, :, 0:1, :]                    # [H, NS, 1, L]
    out = (num / den).reshape(H, B, N, D, L).transpose(1, 2, 4, 0, 3)
    return np.ascontiguousarray(out)


# revision 33
# speedup vs baseline: 1.1961x; 1.0175x over previous
"""Multi-head attention on 8 TRN2 NeuronCores.

Problem: queries [B,N,L,H,E], keys [B,N,S,H,E], values [B,N,S,H,D]
         out[b,n,l,h,:] = softmax(Q[b,n,l,h,:] @ K[b,n,:,h,:]^T / sqrt(E)) @ V[b,n,:,h,:]
with B,N,L,S,H,E,D = 4,7,512,512,8,64,64.

Sharding: head-parallel - core c computes all B*N=28 (b,n) slices for head h=c.

Steady state is ScalarE-paced (exp of 512x512 scores = 2048 elem/lane/slice at
1.2GHz ~= the PE's 8 matmuls at 2.4GHz). Design per slice (fp16 operands):
  1. scoresT chunks [128s, 512l] = K_c^T x Q^T into TWO 2-bank PSUM tiles per
     slice (psA = chunks 0,1; psB = chunks 2,3), tags ping-pong i%2. Tile's
     dependency tracking is per-TILE: with one 4-bank tile, QK(i) waits on the
     copy-out of slice i-2's PV result inside the same tile, serializing
     ACT->PV->copy->QK (~3.3us > the 2.2us ScalarE window) and HAM-oscillating
     the PE. Split tiles break the chain: QK(i+2)'s psA half only waits on
     ACT_A(i); the psB chain has ~0.7us slack over its 2-window budget.
     K chunks are packed alternately on partition halves (c0,c2 on 0:64 /
     c1,c3 on 64:128, Q duplicated on both halves) so consecutive QK matmuls
     hit different PE row groups: LDWEIGHTS overlaps and pairs run
     concurrently in the array (verified: ~0ns start deltas in trace).
  2. exp via two ACTIVATEs per slice: ACT_A (psA) first, ACT_B (psB) second.
  3. po [65, 512] += VA_c^T x attnT_c accumulated into psB's first bank (free
     after ACT_B read). VA = [ones | V] per 128-chunk, so row 0 of po is the
     softmax denominator, rows 1:65 the numerator^T. No on-device
     normalization: ship [denom | num] and divide on host - this removes the
     recip/broadcast/mul chain and its ~15us serialized drain tail.
  4. DVE copies po to SBUF fp16; output DMA goes out on the GpSimd queue so it
     never parks input DMAs on the Sync queue.
"""

import numpy as np

B, N, L, S, H, E, D = 4, 7, 512, 512, 8, 64, 64
NS = B * N          # 28 slices per core
P = 128
SC = S // P         # 4 s-chunks
SCALE = 1.0 / float(np.sqrt(E))

# input tile layout per slice: [128, 1028] =
#   [0:512)     Q^T duplicated: rows 0:64 = [E, L], rows 64:128 = same
#   [512:768)   K^T 2x2: (row 64*(c%2), col 512+128*(c//2)) = chunk c [E, 128]
#   [768:1028)  VA65: 4 chunks x [128, 65] = [ones | V chunk]
QOFF, KOFF, VOFF = 0, 512, 768
TW = 1028
OW = 65             # output rows: 1 denom + 64 numerator

NP = NS // 2        # 14 slice-pairs (DMA batching granularity)
PW = 2 * TW         # 2056 cols per pair tile

_CACHE = {}


def _build_program():
    import concourse.mybir as mybir
    import concourse.tile as tile
    from concourse import bacc
    import concourse.bass as bass

    f32 = mybir.dt.float32
    f16 = mybir.dt.float16
    Exp = mybir.ActivationFunctionType.Exp

    nc = bacc.Bacc("TRN2", target_bir_lowering=False, debug=False)
    inp = nc.dram_tensor("inp", [NP, P, PW], f16, kind="ExternalInput").ap()
    o = nc.dram_tensor("o", [NP, OW, 2 * L], f16, kind="ExternalOutput").ap()

    with tile.TileContext(nc) as tc:
        with (
            tc.tile_pool(name="inpool", bufs=1) as in_pool,
            tc.tile_pool(name="attn", bufs=1) as at_pool,
            tc.tile_pool(name="osb", bufs=1) as osb_pool,
            tc.tile_pool(name="dum", bufs=1) as dum_pool,
            tc.tile_pool(name="ps", bufs=1, space=bass.MemorySpace.PSUM) as ps_pool,
        ):
            # HAM warm-up: ~4.5us of continuous PE busy on a memset tile opens
            # the clock gate (1.2 -> 2.4 GHz) before the steady pipeline.
            warm = in_pool.tile([P, L], f16, tag="warm")
            nc.vector.memset(warm[:], 1.0)
            # Preload the exp table set so the first real ACTIVATE doesn't pay
            # the ~2.7us table load mid-pipeline.
            dummy = dum_pool.tile([1, 2], f32, tag="d0")
            nc.scalar.activation(dummy[:], warm[0:1, 0:2], Exp, scale=SCALE)
            wps = ps_pool.tile([P, 1024], f32, tag="psA0")
            for _ in range(7):
                nc.tensor.matmul(
                    wps[:, 0:L], lhsT=warm[:, 0:P], rhs=warm[:], start=True, stop=True
                )

            in_tiles = {}

            def load(p):
                if p < NP and p not in in_tiles:
                    t = in_pool.tile([P, PW], f16, tag=f"t{p % 5}")
                    nc.sync.dma_start(t[:], inp[p])
                    in_tiles[p] = t

            for p in range(4):
                load(p)

            osb_tiles = {}

            def emit_pv_out(state):
                i, in_t, at, psb = state
                jo = (i % 2) * TW
                # PV accumulates into psB's first bank, free after ACT_B read
                # it. [ones | V] stationary: po row 0 = denominator, rows
                # 1:65 = numerator^T. (Splitting each chunk into concurrent
                # 64-row half-matmuls measured WORSE - interleaved
                # accumulation groups serialize with ~800ns stalls.)
                for c in range(SC):
                    atc = at[c] if isinstance(at, list) else at[:, c * L:(c + 1) * L]
                    nc.tensor.matmul(
                        psb[0:OW, 0:L],
                        lhsT=in_t[:, jo + VOFF + c * OW: jo + VOFF + (c + 1) * OW],
                        rhs=atc,
                        start=(c == 0),
                        stop=(c == SC - 1),
                    )
                # Outputs batch per pair: both slices copy into one osb tile,
                # one DMA per pair (fewer sync events and DMA descriptors).
                if i % 2 == 0:
                    osb_tiles[i // 2] = osb_pool.tile(
                        [OW, 2 * L], f16, tag=f"o{(i // 2) % 3}", name=f"osb{i // 2}"
                    )
                osb = osb_tiles[i // 2]
                nc.vector.tensor_scalar_mul(
                    osb[:, (i % 2) * L:(i % 2 + 1) * L], psb[0:OW, 0:L], 1.0
                )
                if i % 2 == 1:
                    nc.gpsimd.dma_start(o[i // 2], osb_tiles.pop(i // 2)[:])

            pend = []
            for i in range(NS):
                load(i // 2 + 4)
                in_t = in_tiles[i // 2] if i % 2 == 0 else in_tiles.pop(i // 2)
                jo = (i % 2) * TW
                psa = ps_pool.tile([P, 1024], f32, tag=f"psA{i % 2}")
                psb = ps_pool.tile([P, 1024], f32, tag=f"psB{i % 2}")
                # Chunk c: K at rows 64*(c%2), col KOFF+128*(c//2); the order
                # alternates row halves so LDWEIGHTS overlaps the running MM
                # and each (even, odd) pair runs concurrently in the array.
                for c in (0, 1, 2, 3):
                    ps = psa if c < 2 else psb
                    ro = 64 * (c % 2)
                    co = jo + KOFF + P * (c // 2)
                    nc.tensor.matmul(
                        ps[:, (c % 2) * L:(c % 2 + 1) * L],
                        lhsT=in_t[ro:ro + E, co:co + P],
                        rhs=in_t[ro:ro + E, jo + QOFF:jo + QOFF + L],
                        start=True,
                        stop=True,
                    )
                if i < NS - 1:
                    at = at_pool.tile([P, 2048], f16, tag=f"at{i % 4}")
                    nc.scalar.activation(at[:, 0:1024], psa[:], Exp, scale=SCALE)
                    nc.scalar.activation(at[:, 1024:2048], psb[:], Exp, scale=SCALE)
                    if i in (0, 2):
                        # Pace ScalarE during the pipeline fill: windows 0-3
                        # run ~40ns/slice faster than the PE's sustainable
                        # pace; without this throttle the PE falls behind and
                        # the pipeline hunts at +220ns/window for slices 5-9.
                        nc.scalar.activation(dummy[:], warm[0:1, 0:2], Exp, scale=SCALE)
                    pend.append((i, in_t, at, psb))
                else:
                    # Last slice: psb's two chunks get separate ACTs into
                    # separate tiles (tile deps are tile-granular) so the PV
                    # matmuls chase the exp chunk-by-chunk, and PV targets
                    # psa (free right after ACT_A) instead of psb (whose
                    # readers finish last) - shortens the drain tail ~0.7us.
                    ata = at_pool.tile([P, 1024], f16, tag="atl0", name="atl0")
                    nc.scalar.activation(ata[:], psa[:], Exp, scale=SCALE)
                    at = [ata[:, 0:L], ata[:, L:2 * L]]
                    for c in (2, 3):
                        atc = at_pool.tile([P, L], f16, tag=f"atl{c}", name=f"atl{c}")
                        nc.scalar.activation(
                            atc[:], psb[:, (c % 2) * L:(c % 2 + 1) * L], Exp, scale=SCALE
                        )
                        at.append(atc[:])
                    pend.append((i, in_t, at, psa))
                if len(pend) > 1:
                    emit_pv_out(pend.pop(0))
            for state in pend:
                emit_pv_out(state)
    nc.compile()
    return nc


def _prep_inputs(queries, keys, values):
    """Pack per-core fp16 inputs. Core c gets head h=c."""
    q = np.asarray(queries, dtype=np.float32)
    k = np.asarray(keys, dtype=np.float32)
    v = np.asarray(values, dtype=np.float32)

    # Q^T per slice [H, NS, E, L], duplicated onto both partition halves
    qt = np.ascontiguousarray(q.transpose(3, 0, 1, 4, 2)).reshape(H, NS, E, L)
    qd = np.concatenate([qt, qt], axis=2)                      # [H, NS, 128, 512]

    # K^T [H, NS, E, S] -> 2x2 chunk grid [H, NS, 128, 256]
    kt = np.ascontiguousarray(k.transpose(3, 0, 1, 4, 2)).reshape(H, NS, E, S)
    kc = kt.reshape(H, NS, E, 2, 2, P)                         # [..., c//2, c%2, 128]
    # karr[rows 64*(c%2), cols 128*(c//2)] = chunk c
    karr = np.ascontiguousarray(
        kc.transpose(0, 1, 4, 2, 3, 5)                         # [H,NS, c%2, E, c//2, 128]
    ).reshape(H, NS, P, 2 * P)

    # VA65: [H, NS, 4, 128, 65] = [ones | V chunk] -> [H, NS, 128, 260]
    vt = v.transpose(3, 0, 1, 2, 4).reshape(H, NS, SC, P, D)
    va = np.zeros((H, NS, SC, P, OW), dtype=np.float32)
    va[..., 0] = 1.0
    va[..., 1:] = vt
    va = np.ascontiguousarray(va.transpose(0, 1, 3, 2, 4)).reshape(H, NS, P, SC * OW)

    inp = np.concatenate([qd, karr, va], axis=-1).astype(np.float16)
    # batch slice pairs side by side: [H, NP, 128, 2*TW]
    inp = np.ascontiguousarray(
        inp.reshape(H, NP, 2, P, TW).transpose(0, 1, 3, 2, 4)
    ).reshape(H, NP, P, PW)
    return [{"inp": inp[c]} for c in range(H)]


def _run(in_maps, trace=False, tmpdir=None):
    from concourse.bass_utils import run_bass_kernel_spmd

    if "nc" not in _CACHE:
        _CACHE["nc"] = _build_program()
    kwargs = {}
    if tmpdir is not None:
        kwargs["tmpdir"] = tmpdir
    return run_bass_kernel_spmd(
        _CACHE["nc"], in_maps, core_ids=list(range(H)), trace=trace, **kwargs
    )


def kernel(queries, keys, values, _trace=False, _results_out=None, _tmpdir=None):
    in_maps = _prep_inputs(queries, keys, values)
    res = _run(in_maps, trace=_trace, tmpdir=_tmpdir)
    if _results_out is not None:
        _results_out.append(res)
    # res.results[c]["o"]: [NP, 65, 2L] fp16 (slice pair side by side):
    # row 0 = denom, rows 1:65 = numerator^T
    raw = np.stack([res.results[c]["o"] for c in range(H)], axis=0).astype(np.float32)
    raw = raw.reshape(H, NP, OW, 2, L).transpose(0, 1, 3, 2, 4).reshape(H, NS, OW, L)
    num = raw[:, :, 1:, :]                     # [H, NS, D, L]
    den = raw[:, :, 0:1, :]                    # [H, NS, 1, L]
    out = (num / den).reshape(H, B, N, D, L).transpose(1, 2, 4, 0, 3)
    return np.ascontiguousarray(out)
